# revision 7
# baseline (speedup 1.0000x reference)
"""DGCN layer (message passing GNN) on 8 Trainium2 NeuronCores via Bass/Tile.

Strategy (matches the dst-sharded hint):
  - Nodes are range-partitioned across the 8 cores (6250 nodes/core).
  - Each core owns every edge whose dst lies in its node range, so the
    segment-sum over dst is fully core-local.
  - h is replicated into each core's HBM at input staging time (this plays
    the role of the all-gather of src features); the per-edge random
    feat[src] read is an on-device indirect-DMA gather.
  - Per-edge coefficient  coef_e = alpha^dist_e * outdeg[src_e]^-1/2  and the
    per-node output scale  s_v = indeg[v]^-3/2  are tiny O(E)/O(N) host
    scalars computed during sharding; all O(E*D) work runs on device.

Device pipeline per core:
  phase 1 (edge aggregation, accumulates agg^T[feat, node] in SBUF):
    for each 128-edge tile: gather G = h[src] (indirect DMA, batched),
    sel = (iota == r_e) * coef_e  (DVE), psum += matmul(lhsT=G, rhs=sel)
    accumulating over a 128-dst-node window; copy the finished window
    column-block into agg^T.
  phase 2: rst[128 nodes, D] = matmul(lhsT=agg^T window, rhs=W),
    * s_v (per-partition broadcast) + bias, DMA out as [nodes, D].
"""

import math

import numpy as np

P = 128
ALPHA = 0.5
N_CORES = 8
GATHER_TILES = 1  # 128-edge tiles per indirect-DMA gather instruction


def _prep_host(h, src, dst, distance, n_cores):
    """Shard edges by dst range; build per-core padded tile arrays."""
    N, D = h.shape
    E = src.shape[0]
    npc = N // n_cores
    n_windows = (npc + P - 1) // P

    src = np.asarray(src).astype(np.int64)
    dst = np.asarray(dst).astype(np.int64)
    distance = np.asarray(distance)

    out_deg = np.bincount(src, minlength=N).astype(np.float64)
    in_deg = np.bincount(dst, minlength=N).astype(np.float64)
    coef_all = (np.float64(ALPHA) ** distance.astype(np.float64)) * (
        out_deg[src] ** -0.5
    )
    s_all = in_deg**-1.5  # applied after the W matmul

    core_of = dst // npc
    ld = dst - core_of * npc
    w_of = ld // P
    r_of = (ld % P).astype(np.float32)

    gw = core_of * n_windows + w_of  # global window id
    n_gw = n_cores * n_windows
    counts = np.bincount(gw, minlength=n_gw)
    T = max(1, int(math.ceil(counts.max() / P)))
    n_cols = n_windows * T

    order = np.argsort(gw, kind="stable")
    sgw = gw[order]
    win_start = np.concatenate([[0], np.cumsum(counts)[:-1]])
    q = np.arange(E, dtype=np.int64) - win_start[sgw]  # pos within window

    core_arr = sgw // n_windows
    w_arr = sgw % n_windows
    j_arr = q // P
    p_arr = q % P
    col_arr = w_arr * T + j_arr

    srcidx = np.zeros((n_cores, P, n_cols), np.int32)
    rofs = np.zeros((n_cores, P, n_cols), np.float32)
    coef = np.zeros((n_cores, P, n_cols), np.float32)
    srcidx[core_arr, p_arr, col_arr] = src[order].astype(np.int32)
    rofs[core_arr, p_arr, col_arr] = r_of[order]
    coef[core_arr, p_arr, col_arr] = coef_all[order].astype(np.float32)

    snode = np.ones((n_cores, P, n_windows), np.float32)
    nodes = np.arange(N, dtype=np.int64)
    nc_ = nodes // npc
    l = nodes - nc_ * npc
    snode[nc_, l % P, l // P] = s_all.astype(np.float32)

    return srcidx, rofs, coef, snode, npc, n_windows, T, n_cols


def _build_nc(N, D, n_windows, T, n_cols):
    import concourse.bass as bass
    import concourse.bacc as bacc
    import concourse.tile as tile
    from concourse import mybir

    f32 = mybir.dt.float32
    i32 = mybir.dt.int32

    # fconst free-dim layout: rofs | coef | iota | wmat | biasf | snode
    ftot = 2 * n_cols + P + D + D + n_windows

    nc = bacc.Bacc(None, target_bir_lowering=False, debug=False)
    h_d = nc.declare_dram_parameter("h", [N, D], f32, isOutput=False)
    src_d = nc.declare_dram_parameter("srcidx", [P, n_cols], i32, isOutput=False)
    fc_d = nc.declare_dram_parameter("fconst", [P, ftot], f32, isOutput=False)
    out_d = nc.declare_dram_parameter("out", [n_windows * P, D], f32, isOutput=True)

    G = GATHER_TILES
    mult = mybir.AluOpType.mult

    with tile.TileContext(nc) as tc:
        with (
            tc.tile_pool(name="singles", bufs=1) as singles,
            tc.tile_pool(name="gbuf", bufs=6) as gpool,
            tc.tile_pool(name="sel", bufs=6) as selpool,
            tc.tile_pool(name="psum", bufs=4, space="PSUM") as psumpool,
            tc.tile_pool(name="psum2", bufs=2, space="PSUM") as psum2pool,
            tc.tile_pool(name="outp", bufs=3) as outpool,
        ):
            src_sb = singles.tile([P, n_cols], i32)
            nc.sync.dma_start(out=src_sb[:], in_=src_d[:])
            fc_sb = singles.tile([P, ftot], f32)
            nc.sync.dma_start(out=fc_sb[:], in_=fc_d[:])

            r_sb = fc_sb[:, 0:n_cols]
            c_sb = fc_sb[:, n_cols : 2 * n_cols]
            o0 = 2 * n_cols
            io_sb = fc_sb[:, o0 : o0 + P]
            w_sb = fc_sb[:, o0 + P : o0 + P + D]
            b_sb = fc_sb[:, o0 + P + D : o0 + P + 2 * D]
            s_sb = fc_sb[:, o0 + P + 2 * D : o0 + P + 2 * D + n_windows]

            agg = singles.tile([P, n_windows * P], f32)  # agg^T [feat, node]

            gcur = None
            ps = None
            for t in range(n_cols):
                if t % G == 0:
                    gt = min(G, n_cols - t)
                    gcur = gpool.tile([P, G * P], f32, tag="g")
                    nc.gpsimd.indirect_dma_start(
                        out=gcur[:, : gt * P],
                        out_offset=None,
                        in_=h_d[:],
                        in_offset=bass.IndirectOffsetOnAxis(
                            ap=src_sb[:, t : t + gt], axis=0
                        ),
                    )
                w, j = divmod(t, T)
                if j == 0:
                    ps = psumpool.tile([P, P], f32)
                sel = selpool.tile([P, P], f32)
                nc.vector.tensor_tensor(
                    out=sel[:],
                    in0=r_sb[:, t : t + 1].to_broadcast([P, P]),
                    in1=io_sb,
                    op=mybir.AluOpType.is_equal,
                )
                nc.vector.tensor_tensor(
                    out=sel[:],
                    in0=sel[:],
                    in1=c_sb[:, t : t + 1].to_broadcast([P, P]),
                    op=mult,
                )
                g_off = (t % G) * P
                nc.tensor.matmul(
                    out=ps[:],
                    lhsT=gcur[:, g_off : g_off + P],
                    rhs=sel[:],
                    start=(j == 0),
                    stop=(j == T - 1),
                )
                if j == T - 1:
                    nc.scalar.copy(out=agg[:, w * P : (w + 1) * P], in_=ps[:])

            for w in range(n_windows):
                ps2 = psum2pool.tile([P, D], f32)
                nc.tensor.matmul(
                    out=ps2[:],
                    lhsT=agg[:, w * P : (w + 1) * P],
                    rhs=w_sb,
                    start=True,
                    stop=True,
                )
                o = outpool.tile([P, D], f32)
                nc.vector.tensor_tensor(
                    out=o[:],
                    in0=ps2[:],
                    in1=s_sb[:, w : w + 1].to_broadcast([P, D]),
                    op=mult,
                )
                nc.vector.tensor_add(out=o[:], in0=o[:], in1=b_sb)
                nc.sync.dma_start(out=out_d[w * P : (w + 1) * P, :], in_=o[:])

    nc.compile()
    return nc


def kernel(h, src, dst, distance, weight, bias, _trace=False):
    from concourse.bass_utils import run_bass_kernel_spmd

    h = np.ascontiguousarray(np.asarray(h, dtype=np.float32))
    weight = np.ascontiguousarray(np.asarray(weight, dtype=np.float32))
    bias = np.asarray(bias, dtype=np.float32)
    N, D = h.shape

    srcidx, rofs, coef, snode, npc, n_windows, T, n_cols = _prep_host(
        h, src, dst, distance, N_CORES
    )

    iota = np.broadcast_to(np.arange(P, dtype=np.float32)[None, :], (P, P))
    biasf = np.broadcast_to(bias[None, :], (P, D))

    nc = _build_nc(N, D, n_windows, T, n_cols)

    in_maps = []
    for c in range(N_CORES):
        fconst = np.concatenate(
            [rofs[c], coef[c], iota, weight, biasf, snode[c]], axis=1
        ).astype(np.float32)
        in_maps.append(
            {
                "h": h,
                "srcidx": np.ascontiguousarray(srcidx[c]),
                "fconst": np.ascontiguousarray(fconst),
            }
        )

    res = run_bass_kernel_spmd(nc, in_maps, list(range(N_CORES)), trace=_trace)

    out = np.empty((N, D), np.float32)
    for c in range(N_CORES):
        out[c * npc : (c + 1) * npc] = res.results[c]["out"][:npc]

    if _trace:
        return out, res
    return out


# revision 22
# speedup vs baseline: 2.1215x; 2.1215x over previous
"""DGCN layer (message passing GNN) on 8 Trainium2 NeuronCores via Bass/Tile.

Strategy (matches the dst-sharded hint):
  - Nodes are range-partitioned across the 8 cores (6250 nodes/core).
  - Each core owns every edge whose dst lies in its node range, so the
    segment-sum over dst is fully core-local.
  - h is replicated into each core's HBM at input staging time (this plays
    the role of the all-gather of src features); the per-edge random
    feat[src] read is an on-device dma_gather (custom SWDGE ucode), run on
    two SWDGE queues so descriptor generation is parallel across Q7 cores.
  - dma_gather indices are int16 (< 32768), so h is addressed as two
    tables (rows [0, 32768) and [32768, N)); each window's edges are
    grouped into lo-table tiles then hi-table tiles.
  - Per-edge coefficient  coef_e = alpha^dist_e * outdeg[src_e]^-1/2  and the
    per-node output scale  s_v = indeg[v]^-3/2  are tiny O(E)/O(N) host
    scalars computed during sharding; all O(E*D) work runs on device.

Device pipeline per core:
  phase 1 (edge aggregation, accumulates agg^T[feat, node] in SBUF):
    per 128-dst-node window: two dma_gathers (lo/hi tables) fetch all the
    window's h[src] rows; per 128-edge tile sel = (iota == r_e) * coef_e
    (DVE), psum += matmul(lhsT=G_tile, rhs=sel) accumulating over the
    window; copy the finished window column-block into agg^T.
  phase 2: rst[128 nodes, D] = matmul(lhsT=agg^T window, rhs=W),
    * s_v (per-partition broadcast) + bias, DMA out as [nodes, D].
"""

import math

import numpy as np

P = 128
ALPHA = 0.5
N_CORES = 8
SPLIT = 32768  # int16 index limit for dma_gather


def _wrap_idx16(flat):
    """dma_gather index layout: entry k -> partition k%16, column k//16,
    replicated across the 8 gpsimd core groups (partitions 16-127)."""
    n = flat.shape[-1]
    assert n % 16 == 0
    cols = n // 16
    w = np.asarray(flat, np.int16).reshape(cols, 16).T  # [16, cols]
    return np.tile(w, (8, 1))  # [128, cols]


def _prep_host(h, src, dst, distance, n_cores):
    """Shard edges by dst range; build per-core padded tile arrays."""
    N, D = h.shape
    E = src.shape[0]
    npc = N // n_cores
    n_windows = (npc + P - 1) // P

    src = np.asarray(src).astype(np.int64)
    dst = np.asarray(dst).astype(np.int64)
    distance = np.asarray(distance)

    out_deg = np.bincount(src, minlength=N).astype(np.float64)
    in_deg = np.bincount(dst, minlength=N).astype(np.float64)
    coef_all = (np.float64(ALPHA) ** distance.astype(np.float64)) * (
        out_deg[src] ** -0.5
    )
    s_all = in_deg**-1.5  # applied after the W matmul

    # Balanced node -> (core, window, slot) assignment: deal nodes (sorted by
    # in-degree) into the n_cores*n_windows bins in rounds; within a round the
    # heaviest hi-degree nodes go to the lightest bins. This equalizes each
    # window's lo/hi edge counts, minimizing the padded tile count T (which is
    # a global max across bins). The host un-permutes output rows at the end.
    n_bins = n_cores * n_windows
    lo_deg = np.bincount(dst[src < SPLIT], minlength=N).astype(np.int64)
    hi_deg = np.bincount(dst[src >= SPLIT], minlength=N).astype(np.int64)
    order_nodes = np.argsort(-(lo_deg + hi_deg), kind="stable")
    node_bin = np.empty(N, np.int64)
    node_slot = np.empty(N, np.int64)
    lo_sum = np.zeros(n_bins, np.int64)
    hi_sum = np.zeros(n_bins, np.int64)
    fill = np.zeros(n_bins, np.int64)
    pos = 0
    rnd = 0
    while pos < N:
        take = min(n_bins, N - pos)
        nodes_r = order_nodes[pos : pos + take]
        nodes_r = nodes_r[np.argsort(-hi_deg[nodes_r], kind="stable")]
        bins_r = np.argsort(hi_sum, kind="stable")[:take]
        node_bin[nodes_r] = bins_r
        node_slot[nodes_r] = fill[bins_r]
        fill[bins_r] += 1
        lo_sum[bins_r] += lo_deg[nodes_r]
        hi_sum[bins_r] += hi_deg[nodes_r]
        pos += take
        rnd += 1
    node_core = node_bin // n_windows
    node_window = node_bin % n_windows

    core_of = node_core[dst]
    w_of = node_window[dst]
    r_of = node_slot[dst].astype(np.float32)
    is_hi = (src >= SPLIT).astype(np.int64)

    # sort edges by (core, window, lo/hi) — stable
    gw = (core_of * n_windows + w_of) * 2 + is_hi
    n_gw = n_cores * n_windows * 2
    counts = np.bincount(gw, minlength=n_gw)
    cl = counts.reshape(n_cores, n_windows, 2)
    T_lo = max(1, int(math.ceil(cl[:, :, 0].max() / P)))
    T_hi = max(1, int(math.ceil(cl[:, :, 1].max() / P)))
    T = T_lo + T_hi
    n_cols = n_windows * T

    order = np.argsort(gw, kind="stable")
    sgw = gw[order]
    win_start = np.concatenate([[0], np.cumsum(counts)[:-1]])
    q = np.arange(E, dtype=np.int64) - win_start[sgw]  # pos within group

    core_arr = sgw // (2 * n_windows)
    w_arr = (sgw // 2) % n_windows
    hi_arr = sgw % 2
    j_arr = q // P + hi_arr * T_lo  # hi tiles come after the lo tiles
    p_arr = q % P
    col_arr = w_arr * T + j_arr

    rofs = np.zeros((n_cores, P, n_cols), np.float32)
    coef = np.zeros((n_cores, P, n_cols), np.float32)
    rofs[core_arr, p_arr, col_arr] = r_of[order]
    coef[core_arr, p_arr, col_arr] = coef_all[order].astype(np.float32)

    # int16 gather indices, padded with 0 (coef 0 nullifies), table-relative
    srcrel = np.zeros((n_cores, P, n_cols), np.int64)
    srcrel[core_arr, p_arr, col_arr] = src[order] - (src[order] >= SPLIT) * SPLIT

    # wrapped idx16: per core, per window: lo block then hi block.
    # Blocks start at 64B-aligned column offsets (32 int16 cols).
    CL, CH = T_lo * 8, T_hi * 8  # int16 cols per window per table
    CLa = (CL + 31) // 32 * 32
    CHa = (CH + 31) // 32 * 32
    idx16 = np.zeros((n_cores, P, n_windows * (CLa + CHa)), np.int16)
    for c in range(n_cores):
        flat = srcrel[c].T  # [n_cols, P]: (tile, lane)
        for w in range(n_windows):
            lo = flat[w * T : w * T + T_lo].reshape(-1)
            hi = flat[w * T + T_lo : (w + 1) * T].reshape(-1)
            base = w * (CLa + CHa)
            idx16[c, :, base : base + CL] = _wrap_idx16(lo)
            idx16[c, :, base + CLa : base + CLa + CH] = _wrap_idx16(hi)

    snode = np.ones((n_cores, P, n_windows), np.float32)
    snode[node_core, node_slot, node_window] = s_all.astype(np.float32)

    # host-side inverse permutation: node v lives at core_out row
    # node_window*128 + node_slot of core node_core
    out_core = node_core
    out_row = node_window * P + node_slot

    return (
        idx16, rofs, coef, snode, out_core, out_row,
        n_windows, T_lo, T_hi, n_cols,
    )


def _build_nc(N, D, n_windows, T_lo, T_hi, n_cols):
    import concourse.bacc as bacc
    import concourse.tile as tile
    from concourse import mybir

    f32 = mybir.dt.float32
    i16 = mybir.dt.int16
    T = T_lo + T_hi
    CL, CH = T_lo * 8, T_hi * 8
    CLa = (CL + 31) // 32 * 32
    CHa = (CH + 31) // 32 * 32

    # fconst free-dim layout: rofs | coef | iota | wmat | biasf | snode
    ftot = 2 * n_cols + P + D + D + n_windows

    nc = bacc.Bacc(
        None, target_bir_lowering=False, debug=False, num_swdge_queues=2
    )
    h_d = nc.declare_dram_parameter("h", [N, D], f32, isOutput=False)
    idx_d = nc.declare_dram_parameter(
        "idx16", [P, n_windows * (CLa + CHa)], i16, isOutput=False
    )
    fc_d = nc.declare_dram_parameter("fconst", [P, ftot], f32, isOutput=False)
    out_d = nc.declare_dram_parameter("out", [n_windows * P, D], f32, isOutput=True)

    mult = mybir.AluOpType.mult

    with tile.TileContext(nc) as tc:
        with (
            tc.tile_pool(name="singles", bufs=1) as singles,
            tc.tile_pool(name="glo", bufs=3) as glopool,
            tc.tile_pool(name="ghi", bufs=3) as ghipool,
            tc.tile_pool(name="sel", bufs=6) as selpool,
            tc.tile_pool(name="psum", bufs=4, space="PSUM") as psumpool,
            tc.tile_pool(name="psum2", bufs=2, space="PSUM") as psum2pool,
            tc.tile_pool(name="outp", bufs=3) as outpool,
        ):
            idx_sb = singles.tile([P, n_windows * (CLa + CHa)], i16)
            tot = n_windows * (CLa + CHa)
            hd = min(2, n_windows) * (CLa + CHa)
            nc.sync.dma_start(out=idx_sb[:, :hd], in_=idx_d[:, :hd])
            if hd < tot:
                nc.sync.dma_start(out=idx_sb[:, hd:], in_=idx_d[:, hd:])
            fc_sb = singles.tile([P, ftot], f32)
            nc.sync.dma_start(out=fc_sb[:], in_=fc_d[:])

            r_sb = fc_sb[:, 0:n_cols]
            c_sb = fc_sb[:, n_cols : 2 * n_cols]
            o0 = 2 * n_cols
            io_sb = fc_sb[:, o0 : o0 + P]
            w_sb = fc_sb[:, o0 + P : o0 + P + D]
            b_sb = fc_sb[:, o0 + P + D : o0 + P + 2 * D]
            s_sb = fc_sb[:, o0 + P + 2 * D : o0 + P + 2 * D + n_windows]

            agg = singles.tile([P, n_windows * P], f32)  # agg^T [feat, node]

            mid = n_windows - n_windows // 3 if n_windows >= 6 else n_windows

            def _phase2(w2):
                ps2 = psum2pool.tile([P, D], f32)
                nc.tensor.matmul(
                    out=ps2[:],
                    lhsT=agg[:, w2 * P : (w2 + 1) * P],
                    rhs=w_sb,
                    start=True,
                    stop=True,
                )
                o = outpool.tile([P, D], f32)
                nc.vector.tensor_tensor(
                    out=o[:],
                    in0=ps2[:],
                    in1=s_sb[:, w2 : w2 + 1].to_broadcast([P, D]),
                    op=mult,
                )
                nc.vector.tensor_add(out=o[:], in0=o[:], in1=b_sb)
                nc.sync.dma_start(out=out_d[w2 * P : (w2 + 1) * P, :], in_=o[:])

            h_lo = h_d[0 : min(SPLIT, N), :]
            hi_base = SPLIT if N > SPLIT else 0
            h_hi = h_d[hi_base:N, :]

            GCH = 8  # tiles per dma_gather (hw limit: <=1024 idxs/inst)
            qctr = 0
            for w in range(n_windows):
                base = w * (CLa + CHa)
                lo_chunks = []
                for k in range((T_lo + GCH - 1) // GCH):
                    nt = min(GCH, T_lo - k * GCH)
                    g = glopool.tile([P, GCH, P], f32, tag="glo")
                    cb = base + k * GCH * 8
                    nc.gpsimd.dma_gather(
                        g[:, :nt, :],
                        h_lo,
                        idx_sb[:, cb : cb + nt * 8],
                        nt * P,
                        nt * P,
                        P,
                        single_packet=False,
                        queue_num=qctr % 2,
                    )
                    qctr += 1
                    lo_chunks.append(g)
                hi_chunks = []
                for k in range((T_hi + GCH - 1) // GCH):
                    nt = min(GCH, T_hi - k * GCH)
                    g = ghipool.tile([P, GCH, P], f32, tag="ghi")
                    cb = base + CLa + k * GCH * 8
                    nc.gpsimd.dma_gather(
                        g[:, :nt, :],
                        h_hi,
                        idx_sb[:, cb : cb + nt * 8],
                        nt * P,
                        nt * P,
                        P,
                        single_packet=False,
                        queue_num=qctr % 2,
                    )
                    qctr += 1
                    hi_chunks.append(g)
                ps = psumpool.tile([P, P], f32)
                for j in range(T):
                    t = w * T + j
                    sel = selpool.tile([P, P], f32)
                    nc.vector.tensor_tensor(
                        out=sel[:],
                        in0=r_sb[:, t : t + 1].to_broadcast([P, P]),
                        in1=io_sb,
                        op=mybir.AluOpType.is_equal,
                    )
                    nc.vector.tensor_tensor(
                        out=sel[:],
                        in0=sel[:],
                        in1=c_sb[:, t : t + 1].to_broadcast([P, P]),
                        op=mult,
                    )
                    if j < T_lo:
                        lhsT = lo_chunks[j // GCH][:, j % GCH, :]
                    else:
                        jh = j - T_lo
                        lhsT = hi_chunks[jh // GCH][:, jh % GCH, :]
                    nc.tensor.matmul(
                        out=ps[:],
                        lhsT=lhsT,
                        rhs=sel[:],
                        start=(j == 0),
                        stop=(j == T - 1),
                    )
                nc.scalar.copy(out=agg[:, w * P : (w + 1) * P], in_=ps[:])

                if w == mid - 1:
                    # mid-stream burst: finish output for the windows already
                    # aggregated, while gathers for the rest continue
                    for w2 in range(mid):
                        _phase2(w2)
            for w2 in range(mid, n_windows):
                _phase2(w2)

    nc.compile()
    return nc


def kernel(h, src, dst, distance, weight, bias, _trace=False):
    from concourse.bass_utils import run_bass_kernel_spmd

    h = np.ascontiguousarray(np.asarray(h, dtype=np.float32))
    weight = np.ascontiguousarray(np.asarray(weight, dtype=np.float32))
    bias = np.asarray(bias, dtype=np.float32)
    N, D = h.shape

    (
        idx16, rofs, coef, snode, out_core, out_row,
        n_windows, T_lo, T_hi, n_cols,
    ) = _prep_host(h, src, dst, distance, N_CORES)

    iota = np.broadcast_to(np.arange(P, dtype=np.float32)[None, :], (P, P))
    biasf = np.broadcast_to(bias[None, :], (P, D))

    nc = _build_nc(N, D, n_windows, T_lo, T_hi, n_cols)

    in_maps = []
    for c in range(N_CORES):
        fconst = np.concatenate(
            [rofs[c], coef[c], iota, weight, biasf, snode[c]], axis=1
        ).astype(np.float32)
        in_maps.append(
            {
                "h": h,
                "idx16": np.ascontiguousarray(idx16[c]),
                "fconst": np.ascontiguousarray(fconst),
            }
        )

    res = run_bass_kernel_spmd(nc, in_maps, list(range(N_CORES)), trace=_trace)

    stacked = np.stack([res.results[c]["out"] for c in range(N_CORES)])
    out = stacked[out_core, out_row].astype(np.float32)

    if _trace:
        return out, res
    return out
